# revision 11
# baseline (speedup 1.0000x reference)
"""GAT (2-layer) Trainium2 Bass kernel, 8-core SPMD.

Strategy (dst-node sharding + per-edge dma_gather):
  - Nodes are dealt to 8 cores by degree (snake deal) and degree-sorted into
    "lanes"; 128-lane blocks with per-block padded slot counts (degree
    bucketing) turn the per-dst segment softmax + weighted aggregation into
    uniform strided DVE ops (partition = dst lane, free = edge slot).
  - Layer tables (node features + exp'd attention terms) are built sharded,
    AllGathered, then per-edge rows are fetched with dma_gather.  int16
    gather indices only span 32768 rows, so each node is colored lo/hi and
    every block does one gather from the lo view [0, 32768) and one from the
    hi view [TAB-32768, TAB).
  - leaky_relu(ls+ld) inside exp factorizes: exp(lrelu(z)) =
    max(exp(ls)exp(ld), exp(.2ls)exp(.2ld)), so softmax runs in the linear
    domain with node-level exps only (P1,P2 per src / Q1,Q2 per dst).
  - Layer 2 aggregates in the 64-dim projected space: h2 = elu(out1) @ W2 is
    computed node-level (PE transpose + matmul), so the second gather rows
    are only 256B.
"""

import numpy as np
import ml_dtypes

import concourse.bacc as bacc
import concourse.mybir as mybir
import concourse.tile as tile

F32 = mybir.dt.float32
BF16 = mybir.dt.bfloat16
I16 = mybir.dt.int16
AF = mybir.ActivationFunctionType
ALU = mybir.AluOpType

NEG_SLOPE = 0.2
P = 128


class Cfg:
    def __init__(self, N, E, in_dim, hid, heads, out_dim, n_cores=8):
        self.N, self.E = N, E
        self.in_dim, self.hid, self.heads, self.out_dim = in_dim, hid, heads, out_dim
        self.n_cores = n_cores
        self.hh = hid * heads                      # layer-1 output channels
        assert N % n_cores == 0
        self.n_real = N // n_cores                 # real lanes per core
        self.L = ((self.n_real + P) // P) * P      # padded lanes (incl dead)
        assert self.n_real < self.L, "need dead lanes for gather padding"
        self.B = self.L // P                       # dst blocks per core
        self.TAB = n_cores * self.L                # table rows
        self.HIB = max(self.TAB - 32768, 0)        # hi-view base row
        self.row1 = (self.hh * 2 + 4 * self.heads * 2 + 255) // 256 * 128
        self.row1 = 384                            # 256 bf16 h + 16 f32 P + pad
        self.row2 = 128                            # 64 bf16 h2 + 2 f32 P + pad
        self.pad_lo = self.n_real                  # core0 dead lane pid
        self.pad_hi = (n_cores - 1) * self.L + self.n_real - self.HIB


FULL = Cfg(50000, 800000, 256, 32, 8, 64)


# --------------------------------------------------------------------------
# host-side graph preprocessing (index manipulation only)
# --------------------------------------------------------------------------

def preprocess(edge_index, cfg: Cfg):
    N, C = cfg.N, cfg.n_cores
    src = np.asarray(edge_index[0], np.int64)
    dst = np.asarray(edge_index[1], np.int64)
    # self-loops are handled residently on-device; edge lists exclude them
    deg = np.bincount(dst, minlength=N)

    # snake-deal nodes to cores by degree; lane = deal round (degree-sorted)
    order = np.argsort(-deg, kind="stable")
    rounds = np.arange(N) // C
    j = np.arange(N) % C
    core_sorted = np.where(rounds % 2 == 0, j, C - 1 - j)
    core_of = np.empty(N, np.int64)
    lane_of = np.empty(N, np.int64)
    core_of[order] = core_sorted
    lane_of[order] = rounds
    pid = core_of * cfg.L + lane_of

    sp = pid[src]
    ecore = core_of[dst]
    eblk = lane_of[dst] // P
    elane = lane_of[dst] % P
    lane_key = (ecore * cfg.B + eblk) * P + elane

    # per-edge side choice: forced lo (sp < HIB), forced hi (sp >= 32768),
    # free in between.  Per lane, fill lo up to a per-lane target that
    # balances the two sides as well as the forced counts allow.
    forced_hi = sp >= 32768
    is_free = (~forced_hi) & (sp >= cfg.HIB)
    NL = C * cfg.B * P
    tot = np.bincount(lane_key, minlength=NL)
    flo = np.bincount(lane_key[(~forced_hi) & (~is_free)], minlength=NL)
    ffr = np.bincount(lane_key[is_free], minlength=NL)
    # per-block optimal (D_lo, D_hi): sweep D_lo, maximize per-lane lo fill,
    # pick the split minimizing total padded slots
    lo_target = np.zeros(NL, np.int64)
    blk_of = (np.arange(NL) // P) % cfg.B
    for b in range(cfg.B):
        sel = blk_of == b
        fl, fr, tt = flo[sel], ffr[sel], tot[sel]
        best = None
        for DLc in range(int(fl.max(initial=0)), int(tt.max(initial=0)) + 1):
            lo = np.minimum(np.minimum(DLc, fl + fr), tt)
            DH = int((tt - lo).max(initial=0))
            if best is None or DLc + DH < best[0]:
                best = (DLc + DH, DLc, lo)
        lo_target[sel] = best[2] if best else 0

    # order edges per lane: forced-lo, free, forced-hi; cum position in lane
    cls = np.where(forced_hi, 2, np.where(is_free, 1, 0))
    o = np.lexsort((cls, lane_key))
    ks = lane_key[o]
    if len(ks):
        first = np.r_[0, np.where(np.diff(ks) != 0)[0] + 1]
        firsts = np.zeros(len(ks), np.int64)
        firsts[first] = first
        np.maximum.accumulate(firsts, out=firsts)
        cum = np.arange(len(ks)) - firsts
    else:
        cum = np.array([], np.int64)
    lo_t_e = lo_target[ks]
    side_hi = cum >= lo_t_e
    slot = np.where(side_hi, cum - lo_t_e, cum)
    sp_o = sp[o]
    assert not (side_hi & (sp_o < cfg.HIB)).any()
    assert not ((~side_hi) & (sp_o >= 32768)).any()

    lo_cnt = lo_target.reshape(C, cfg.B, P)
    hi_cnt = (tot - lo_target).reshape(C, cfg.B, P)
    D_lo = lo_cnt.max(axis=(0, 2)).astype(np.int64)
    D_hi = hi_cnt.max(axis=(0, 2)).astype(np.int64)

    def build_side(hi_side, D, pad_idx, base):
        m = side_hi == hi_side
        ksm = ks[m]
        cores_s = ksm // (cfg.B * P)
        blks_s = (ksm // P) % cfg.B
        lanes_s = ksm % P
        vals = sp_o[m] - base
        assert len(vals) == 0 or (vals.min() >= 0 and vals.max() < 32768)
        widths = 8 * D + 1  # int16 columns (of 16) per gather call
        offs = np.r_[0, np.cumsum(widths)].astype(np.int64)
        WT = int(offs[-1])
        out = []
        for c in range(C):
            arr = np.full((16, WT), pad_idx, np.int64)
            mm = cores_s == c
            flat = offs[blks_s[mm]] * 16 + slot[m][mm] * P + lanes_s[mm]
            arr[flat % 16, flat // 16] = vals[mm]
            out.append(np.tile(arr, (8, 1)).astype(np.int16))
        return out, offs

    idx_lo, offs_lo = build_side(False, D_lo, cfg.pad_lo, 0)
    idx_hi, offs_hi = build_side(True, D_hi, cfg.pad_hi, cfg.HIB)

    perm = np.full(C * cfg.L, -1, np.int64)
    perm[pid] = np.arange(N)

    return dict(
        pid=pid, perm=perm, D_lo=D_lo, D_hi=D_hi,
        idx_lo=idx_lo, idx_hi=idx_hi, offs_lo=offs_lo, offs_hi=offs_hi,
    )


# --------------------------------------------------------------------------
# device program
# --------------------------------------------------------------------------

def build_program(cfg: Cfg, prep):
    c = cfg
    D_lo, D_hi = prep["D_lo"], prep["D_hi"]
    offs_lo, offs_hi = prep["offs_lo"], prep["offs_hi"]
    WLO, WHI = int(offs_lo[-1]), int(offs_hi[-1])
    HH, OD = c.hh, c.out_dim
    HE, HD = c.heads, c.hid

    nc = bacc.Bacc("TRN2", num_swdge_queues=2)
    xT = nc.dram_tensor("xT", [c.in_dim, c.L], F32, kind="ExternalInput")
    wpack = nc.dram_tensor("wpack", [c.in_dim, HH + 2 * HE], F32, kind="ExternalInput")
    w2bf = nc.dram_tensor("w2bf", [HH, OD], BF16, kind="ExternalInput")
    b1b = nc.dram_tensor("b1b", [P, HH], F32, kind="ExternalInput")
    v2u2 = nc.dram_tensor("v2u2", [P, 2 * HH], F32, kind="ExternalInput")
    b2b = nc.dram_tensor("b2b", [P, OD], F32, kind="ExternalInput")
    ident = nc.dram_tensor("ident", [P, P], F32, kind="ExternalInput")
    mask_d = nc.dram_tensor("mask1", [P, 1], F32, kind="ExternalInput")
    madd_d = nc.dram_tensor("madd1", [P, 1], F32, kind="ExternalInput")
    ixlo_d = nc.dram_tensor("idx_lo", [P, WLO], I16, kind="ExternalInput")
    ixhi_d = nc.dram_tensor("idx_hi", [P, WHI], I16, kind="ExternalInput")
    out_d = nc.dram_tensor("out", [c.L, OD], F32, kind="ExternalOutput")

    t1s = nc.dram_tensor("t1s", [c.L, c.row1], BF16)
    T1 = nc.dram_tensor("T1", [c.TAB, c.row1], BF16, addr_space="Shared")
    t2s = nc.dram_tensor("t2s", [c.L, c.row2], BF16)
    T2 = nc.dram_tensor("T2", [c.TAB, c.row2], BF16, addr_space="Shared")

    KT = c.in_dim // P  # contraction tiles (2)
    NPACK = HH + 2 * HE  # 272

    with tile.TileContext(nc) as tc:
        with (
            tc.tile_pool(name="res", bufs=1) as res,
            tc.tile_pool(name="pmm", bufs=2, space="PSUM") as pmm,
            tc.tile_pool(name="ptp", bufs=2, space="PSUM") as ptp,
            tc.tile_pool(name="ph2", bufs=2, space="PSUM") as ph2p,
            tc.tile_pool(name="t1p", bufs=3) as t1p,
        ):
            # ---- resident loads
            wps = res.tile([P, KT, NPACK], F32)
            nc.sync.dma_start(out=wps[:], in_=wpack.ap().rearrange("(k p) n -> p k n", p=P))
            w2s = res.tile([P, KT, OD], BF16)
            nc.sync.dma_start(out=w2s[:], in_=w2bf.ap().rearrange("(k p) n -> p k n", p=P))
            b1s = res.tile([P, HH], F32)
            nc.sync.dma_start(out=b1s[:], in_=b1b.ap())
            vus = res.tile([P, 2, HH], F32)
            nc.sync.dma_start(out=vus[:], in_=v2u2.ap().rearrange("p (t n) -> p t n", n=HH))
            b2s = res.tile([P, OD], F32)
            nc.sync.dma_start(out=b2s[:], in_=b2b.ap())
            ids = res.tile([P, P], F32)
            nc.sync.dma_start(out=ids[:], in_=ident.ap())
            msks = res.tile([P, 1], F32)
            nc.sync.dma_start(out=msks[:], in_=mask_d.ap())
            mads = res.tile([P, 1], F32)
            nc.sync.dma_start(out=mads[:], in_=madd_d.ap())
            ixlo = res.tile([P, WLO], I16)
            nc.sync.dma_start(out=ixlo[:], in_=ixlo_d.ap())
            ixhi = res.tile([P, WHI], I16)
            nc.sync.dma_start(out=ixhi[:], in_=ixhi_d.ap())
            q12 = res.tile([P, c.B, 2 * HE], F32)    # layer-1 Q1|Q2 per block
            q2r = res.tile([P, c.B, 2], F32)         # layer-2 Q1|Q2
            hres = res.tile([P, c.B, HH], BF16)      # own h rows (self-loops)
            p12r = res.tile([P, c.B, 2 * HE], F32)   # own P1|P2
            h2res = res.tile([P, c.B, OD], BF16)     # own h2 rows
            p2r = res.tile([P, c.B, 2], F32)         # own layer-2 P1|P2

            # ================= phase 0: node tables =================
            with tc.tile_pool(name="p0res", bufs=1) as p0res:
                xts = p0res.tile([P, KT, c.L], F32)
                nc.sync.dma_start(out=xts[:], in_=xT.ap().rearrange("(k p) l -> p k l", p=P))
                for b in range(c.B):
                    ps = pmm.tile([P, NPACK], F32, tag="p0")
                    for k in range(KT):
                        nc.tensor.matmul(
                            ps[:], xts[:, k, b * P:(b + 1) * P], wps[:, k, :],
                            start=(k == 0), stop=(k == KT - 1),
                        )
                    t1t = t1p.tile([P, c.row1], BF16, tag="t1")
                    t1f = t1t[:].bitcast(F32)  # [P, row1//2]
                    hw1 = HH // 2  # f32 col of P1
                    nc.vector.tensor_copy(out=t1t[:, 0:HH], in_=ps[:, 0:HH])
                    nc.scalar.activation(t1f[:, hw1:hw1 + HE], ps[:, HH:HH + HE], AF.Exp)
                    nc.scalar.activation(t1f[:, hw1 + HE:hw1 + 2 * HE], ps[:, HH:HH + HE],
                                         AF.Exp, scale=NEG_SLOPE)
                    nc.scalar.activation(q12[:, b, 0:HE], ps[:, HH + HE:HH + 2 * HE], AF.Exp)
                    nc.scalar.activation(q12[:, b, HE:2 * HE], ps[:, HH + HE:HH + 2 * HE],
                                         AF.Exp, scale=NEG_SLOPE)
                    if b == c.B - 1:
                        nc.vector.tensor_tensor(
                            out=t1f[:, hw1:hw1 + 2 * HE], in0=t1f[:, hw1:hw1 + 2 * HE],
                            in1=msks[:].to_broadcast([P, 2 * HE]), op=ALU.mult)
                    nc.vector.tensor_copy(out=hres[:, b, :], in_=ps[:, 0:HH])
                    nc.vector.tensor_copy(out=p12r[:, b, :], in_=t1f[:, hw1:hw1 + 2 * HE])
                    nc.sync.dma_start(out=t1s[b * P:(b + 1) * P, :], in_=t1t[:])

            # ---- AllGather layer-1 table
            nc.gpsimd.collective_compute(
                "AllGather", ALU.bypass,
                replica_groups=[list(range(c.n_cores))],
                ins=[t1s.ap()], outs=[T1.ap()],
            )

            with (
                tc.tile_pool(name="gat", bufs=3) as gat,
                tc.tile_pool(name="gat2", bufs=2) as gat2,
                tc.tile_pool(name="work", bufs=2) as wk,
                tc.tile_pool(name="outp", bufs=3) as outp,
            ):
                def gather(tag, view, ix, off, D, row, qn, pool=None):
                    # chunk calls to <=1024 idx so every call is single_packet
                    g = (pool or gat).tile([P, D + 1, row], BF16, tag=tag)
                    c0 = 0
                    while c0 < D:
                        dc = min(7, D - c0)
                        last = c0 + dc == D
                        ext = 1 if last else 0
                        n = P * dc + 16 * ext
                        nc.gpsimd.dma_gather(
                            g[:, c0:c0 + dc + ext, :], view,
                            ix[:, off + 8 * c0:off + 8 * (c0 + dc) + ext], n, n, row,
                            single_packet=True, queue_num=qn,
                        )
                        c0 += dc
                    return g

                def u_chain(g, b, D, hw, qtile, nq, tagp):
                    """u weights [P, D, nq, 1] bf16 from gathered P and resident Q."""
                    gf = g[:].bitcast(F32)
                    ut = wk.tile([P, D, 2 * nq], F32, tag=f"ut{tagp}")
                    nc.vector.tensor_tensor(
                        out=ut[:], in0=gf[:, 0:D, hw:hw + 2 * nq],
                        in1=qtile[:, b:b + 1, :].to_broadcast([P, D, 2 * nq]),
                        op=ALU.mult)
                    u = wk.tile([P, D, nq, 1], BF16, tag=f"u{tagp}")
                    nc.vector.tensor_tensor(
                        out=u[:, :, :, 0], in0=ut[:, :, 0:nq], in1=ut[:, :, nq:2 * nq],
                        op=ALU.max)
                    return u

                # ================= phase 1: layer-1 edges =================
                t1lo = T1[0:min(c.TAB, 32768), :]
                t1hi = T1[c.HIB:c.TAB, :]
                for b in range(c.B):
                    sides = []
                    for sd, (D, off, view, ix, qn) in enumerate([
                        (int(D_lo[b]), int(offs_lo[b]), t1lo, ixlo, 0),
                        (int(D_hi[b]), int(offs_hi[b]), t1hi, ixhi, 1),
                    ]):
                        if D == 0:
                            continue
                        g = gather(f"g{sd}", view, ix, off, D, c.row1, qn)
                        u = u_chain(g, b, D, HH // 2, q12, HE, sd)
                        nc.vector.tensor_tensor(
                            out=g[:, 0:D, 0:HH].rearrange("p d (h ch) -> p d h ch", ch=HD),
                            in0=u[:].to_broadcast([P, D, HE, HD]),
                            in1=g[:, 0:D, 0:HH].rearrange("p d (h ch) -> p d h ch", ch=HD),
                            op=ALU.mult)
                        sp_ = wk.tile([P, HE], F32, tag=f"sp{sd}")
                        nc.vector.tensor_reduce(
                            out=sp_[:], in_=u[:].rearrange("p d h o -> p h (d o)"),
                            axis=mybir.AxisListType.X, op=ALU.add)
                        ap_ = wk.tile([P, HH], F32, tag=f"ap{sd}")
                        nc.vector.tensor_reduce(
                            out=ap_[:], in_=g[:, 0:D, 0:HH].rearrange("p d n -> p n d"),
                            axis=mybir.AxisListType.X, op=ALU.add)
                        sides.append((sp_, ap_))
                    s = wk.tile([P, HE], F32, tag="s")
                    agg = wk.tile([P, HH], F32, tag="agg")
                    if len(sides) == 2:
                        nc.vector.tensor_tensor(out=s[:], in0=sides[0][0][:], in1=sides[1][0][:], op=ALU.add)
                        nc.vector.tensor_tensor(out=agg[:], in0=sides[0][1][:], in1=sides[1][1][:], op=ALU.add)
                    elif len(sides) == 1:
                        nc.vector.tensor_copy(out=s[:], in_=sides[0][0][:])
                        nc.vector.tensor_copy(out=agg[:], in_=sides[0][1][:])
                    else:
                        nc.vector.memset(s[:], 0)
                        nc.vector.memset(agg[:], 0)
                    # self-loop contribution (resident, no gather)
                    usl = wk.tile([P, 2 * HE], F32, tag="usl")
                    nc.vector.tensor_tensor(out=usl[:], in0=p12r[:, b, :], in1=q12[:, b, :], op=ALU.mult)
                    us = wk.tile([P, HE, 1], F32, tag="us")
                    nc.vector.tensor_tensor(out=us[:, :, 0], in0=usl[:, 0:HE], in1=usl[:, HE:2 * HE], op=ALU.max)
                    nc.vector.tensor_tensor(out=s[:], in0=s[:], in1=us[:, :, 0], op=ALU.add)
                    nc.vector.tensor_tensor(
                        out=hres[:, b, :].rearrange("p (h ch) -> p h ch", ch=HD),
                        in0=us[:].to_broadcast([P, HE, HD]),
                        in1=hres[:, b, :].rearrange("p (h ch) -> p h ch", ch=HD), op=ALU.mult)
                    nc.vector.tensor_tensor(out=agg[:], in0=agg[:], in1=hres[:, b, :], op=ALU.add)

                    if b == c.B - 1:
                        nc.vector.tensor_tensor(
                            out=s[:], in0=s[:],
                            in1=mads[:].to_broadcast([P, HE]), op=ALU.add)
                    rec = wk.tile([P, HE, 1], F32, tag="rec")
                    nc.vector.reciprocal(rec[:, :, 0], s[:])
                    o1 = wk.tile([P, HH], F32, tag="o1")
                    nc.vector.tensor_tensor(
                        out=o1[:].rearrange("p (h ch) -> p h ch", ch=HD),
                        in0=agg[:].rearrange("p (h ch) -> p h ch", ch=HD),
                        in1=rec[:].to_broadcast([P, HE, HD]), op=ALU.mult)
                    nc.vector.tensor_tensor(out=o1[:], in0=o1[:], in1=b1s[:], op=ALU.add)
                    # elu(o1) = relu(o1) + exp(o1 - relu(o1)) - 1
                    rl = wk.tile([P, HH], F32, tag="rl")
                    nc.scalar.activation(rl[:], o1[:], AF.Relu)
                    ng = wk.tile([P, HH], F32, tag="ng")
                    nc.vector.tensor_tensor(out=ng[:], in0=o1[:], in1=rl[:], op=ALU.subtract)
                    nc.scalar.activation(ng[:], ng[:], AF.Exp)
                    el = wk.tile([P, 1, HH], F32, tag="el")
                    nc.vector.tensor_tensor(out=el[:, 0, :], in0=rl[:], in1=ng[:], op=ALU.add)
                    nc.vector.tensor_scalar_add(el[:, 0, :], el[:, 0, :], -1.0)
                    # ls2 / ld2
                    d2 = wk.tile([P, 2, HH], F32, tag="d2")
                    nc.vector.tensor_tensor(
                        out=d2[:], in0=el[:].to_broadcast([P, 2, HH]), in1=vus[:], op=ALU.mult)
                    ll = wk.tile([P, 2], F32, tag="ll")
                    nc.vector.tensor_reduce(out=ll[:], in_=d2[:], axis=mybir.AxisListType.X,
                                            op=ALU.add)
                    t2t = t1p.tile([P, c.row2], BF16, tag="t2")
                    t2f = t2t[:].bitcast(F32)
                    h2w = OD // 2
                    nc.scalar.activation(t2f[:, h2w:h2w + 1], ll[:, 0:1], AF.Exp)
                    nc.scalar.activation(t2f[:, h2w + 1:h2w + 2], ll[:, 0:1], AF.Exp,
                                         scale=NEG_SLOPE)
                    nc.scalar.activation(q2r[:, b, 0:1], ll[:, 1:2], AF.Exp)
                    nc.scalar.activation(q2r[:, b, 1:2], ll[:, 1:2], AF.Exp, scale=NEG_SLOPE)
                    # h2 = el @ W2 (PE transpose path)
                    ph2 = ph2p.tile([P, OD], F32, tag="ph2")
                    for k in range(KT):
                        tp = ptp.tile([P, P], F32, tag="tp")
                        nc.tensor.transpose(tp[:], el[:, 0, k * P:(k + 1) * P], ids[:])
                        eb = wk.tile([P, P], BF16, tag="eb")
                        nc.vector.tensor_copy(out=eb[:], in_=tp[:])
                        nc.tensor.matmul(ph2[:], eb[:], w2s[:, k, :],
                                         start=(k == 0), stop=(k == KT - 1))
                    nc.vector.tensor_copy(out=t2t[:, 0:OD], in_=ph2[:])
                    if b == c.B - 1:
                        nc.vector.tensor_tensor(
                            out=t2t[:], in0=t2t[:],
                            in1=msks[:].to_broadcast([P, c.row2]), op=ALU.mult)
                    nc.vector.tensor_copy(out=h2res[:, b, :], in_=t2t[:, 0:OD])
                    nc.vector.tensor_copy(out=p2r[:, b, :], in_=t2f[:, h2w:h2w + 2])
                    nc.sync.dma_start(out=t2s[b * P:(b + 1) * P, :], in_=t2t[:])

                # ---- AllGather layer-2 table
                nc.gpsimd.collective_compute(
                    "AllGather", ALU.bypass,
                    replica_groups=[list(range(c.n_cores))],
                    ins=[t2s.ap()], outs=[T2.ap()],
                )

                # ================= phase 2: layer-2 edges =================
                t2lo = T2[0:min(c.TAB, 32768), :]
                t2hi = T2[c.HIB:c.TAB, :]
                for b in range(c.B):
                    sides = []
                    for sd, (D, off, view, ix, qn) in enumerate([
                        (int(D_lo[b]), int(offs_lo[b]), t2lo, ixlo, 0),
                        (int(D_hi[b]), int(offs_hi[b]), t2hi, ixhi, 1),
                    ]):
                        if D == 0:
                            continue
                        g = gather(f"h{sd}", view, ix, off, D, c.row2, qn, pool=gat2)
                        u = u_chain(g, b, D, OD // 2, q2r, 1, 2 + sd)
                        nc.vector.tensor_tensor(
                            out=g[:, 0:D, 0:OD],
                            in0=u[:, :, 0, :].to_broadcast([P, D, OD]),
                            in1=g[:, 0:D, 0:OD], op=ALU.mult)
                        sp_ = wk.tile([P, 1], F32, tag=f"s2p{sd}")
                        nc.vector.tensor_reduce(
                            out=sp_[:], in_=u[:].rearrange("p d h o -> p h (d o)"),
                            axis=mybir.AxisListType.X, op=ALU.add)
                        ap_ = wk.tile([P, OD], F32, tag=f"a2p{sd}")
                        nc.vector.tensor_reduce(
                            out=ap_[:], in_=g[:, 0:D, 0:OD].rearrange("p d n -> p n d"),
                            axis=mybir.AxisListType.X, op=ALU.add)
                        sides.append((sp_, ap_))
                    s = wk.tile([P, 1], F32, tag="s2")
                    agg = wk.tile([P, OD], F32, tag="agg2")
                    if len(sides) == 2:
                        nc.vector.tensor_tensor(out=s[:], in0=sides[0][0][:], in1=sides[1][0][:], op=ALU.add)
                        nc.vector.tensor_tensor(out=agg[:], in0=sides[0][1][:], in1=sides[1][1][:], op=ALU.add)
                    elif len(sides) == 1:
                        nc.vector.tensor_copy(out=s[:], in_=sides[0][0][:])
                        nc.vector.tensor_copy(out=agg[:], in_=sides[0][1][:])
                    else:
                        nc.vector.memset(s[:], 0)
                        nc.vector.memset(agg[:], 0)
                    usl2 = wk.tile([P, 2], F32, tag="usl2")
                    nc.vector.tensor_tensor(out=usl2[:], in0=p2r[:, b, :], in1=q2r[:, b, :], op=ALU.mult)
                    us2 = wk.tile([P, 1], F32, tag="us2")
                    nc.vector.tensor_tensor(out=us2[:], in0=usl2[:, 0:1], in1=usl2[:, 1:2], op=ALU.max)
                    nc.vector.tensor_tensor(out=s[:], in0=s[:], in1=us2[:], op=ALU.add)
                    nc.vector.tensor_tensor(
                        out=h2res[:, b, :], in0=us2[:].to_broadcast([P, OD]),
                        in1=h2res[:, b, :], op=ALU.mult)
                    nc.vector.tensor_tensor(out=agg[:], in0=agg[:], in1=h2res[:, b, :], op=ALU.add)
                    if b == c.B - 1:
                        nc.vector.tensor_tensor(
                            out=s[:], in0=s[:], in1=mads[:], op=ALU.add)
                    rec = wk.tile([P, 1], F32, tag="rec2")
                    nc.vector.reciprocal(rec[:], s[:])
                    o2 = outp.tile([P, OD], F32, tag="o2")
                    nc.vector.tensor_tensor(
                        out=o2[:], in0=agg[:],
                        in1=rec[:].to_broadcast([P, OD]), op=ALU.mult)
                    nc.vector.tensor_tensor(out=o2[:], in0=o2[:], in1=b2s[:], op=ALU.add)
                    nc.sync.dma_start(out=out_d[b * P:(b + 1) * P, :], in_=o2[:])

    nc.compile()
    return nc


# --------------------------------------------------------------------------
# host wrapper
# --------------------------------------------------------------------------

def make_inputs(x, W1, a_src1, a_dst1, b1, W2, a_src2, a_dst2, b2, cfg, prep):
    c = cfg
    HH, HE, HD = c.hh, c.heads, c.hid
    A = np.zeros((HH, HE), np.float32)
    Ad = np.zeros((HH, HE), np.float32)
    for h in range(HE):
        A[h * HD:(h + 1) * HD, h] = a_src1[h]
        Ad[h * HD:(h + 1) * HD, h] = a_dst1[h]
    U1 = (W1 @ A).astype(np.float32)
    V1 = (W1 @ Ad).astype(np.float32)
    wpack = np.concatenate([W1.astype(np.float32), U1, V1], axis=1)
    v2 = (W2 @ a_src2[0]).astype(np.float32)
    u2 = (W2 @ a_dst2[0]).astype(np.float32)
    v2u2 = np.tile(np.concatenate([v2, u2])[None, :], (P, 1)).astype(np.float32)
    b1b = np.tile(b1[None, :], (P, 1)).astype(np.float32)
    b2b = np.tile(b2[None, :], (P, 1)).astype(np.float32)
    ident = np.eye(P, dtype=np.float32)
    mask1 = np.ones((P, 1), np.float32)
    d0 = c.n_real % P
    mask1[d0:, 0] = 0.0
    madd1 = 1.0 - mask1
    w2bf = W2.astype(ml_dtypes.bfloat16)

    perm = prep["perm"]
    in_maps = []
    for k in range(c.n_cores):
        pk = perm[k * c.L:(k + 1) * c.L]
        xs = np.zeros((c.L, c.in_dim), np.float32)
        real = pk >= 0
        xs[real] = x[pk[real]]
        in_maps.append({
            "xT": np.ascontiguousarray(xs.T),
            "wpack": wpack, "w2bf": w2bf, "b1b": b1b, "v2u2": v2u2,
            "b2b": b2b, "ident": ident, "mask1": mask1, "madd1": madd1,
            "idx_lo": prep["idx_lo"][k], "idx_hi": prep["idx_hi"][k],
        })
    return in_maps


def unshard(results, cfg, prep):
    c = cfg
    full = np.concatenate([r["out"] for r in results], axis=0)  # [C*L, OD]
    return full[prep["pid"]].astype(np.float32)


_CACHE = {}


def run(inputs, cfg=FULL):
    edge_index = np.asarray(inputs["edge_index"])
    if cfg.N not in _CACHE:
        prep = preprocess(edge_index, cfg)
        nc = build_program(cfg, prep)
        _CACHE[cfg.N] = (prep, nc)
    prep, nc = _CACHE[cfg.N]
    in_maps = make_inputs(
        np.asarray(inputs["x"], np.float32), np.asarray(inputs["W1"], np.float32),
        np.asarray(inputs["a_src1"], np.float32), np.asarray(inputs["a_dst1"], np.float32),
        np.asarray(inputs["b1"], np.float32), np.asarray(inputs["W2"], np.float32),
        np.asarray(inputs["a_src2"], np.float32), np.asarray(inputs["a_dst2"], np.float32),
        np.asarray(inputs["b2"], np.float32), cfg, prep)
    from concourse.bass_utils import run_bass_kernel_spmd
    last = None
    for attempt in range(3):
        try:
            res = run_bass_kernel_spmd(nc, in_maps, core_ids=list(range(cfg.n_cores)))
            return unshard(res.results, cfg, prep)
        except Exception as e:  # transient tunnel/device hiccups
            last = e
            import time as _t
            _t.sleep(10)
    raise last


def kernel(**inputs) -> np.ndarray:
    return run(inputs, FULL)


# revision 12
# speedup vs baseline: 1.4057x; 1.4057x over previous
"""GAT (2-layer) Trainium2 Bass kernel, 8-core SPMD.

Strategy (dst-node sharding + per-edge dma_gather):
  - Nodes are dealt to 8 cores by degree (snake deal) and degree-sorted into
    "lanes"; 128-lane blocks with per-block padded slot counts (degree
    bucketing) turn the per-dst segment softmax + weighted aggregation into
    uniform strided DVE ops (partition = dst lane, free = edge slot).
  - Layer tables (node features + exp'd attention terms) are built sharded,
    AllGathered, then per-edge rows are fetched with dma_gather.  int16
    gather indices only span 32768 rows, so each node is colored lo/hi and
    every block does one gather from the lo view [0, 32768) and one from the
    hi view [TAB-32768, TAB).
  - leaky_relu(ls+ld) inside exp factorizes: exp(lrelu(z)) =
    max(exp(ls)exp(ld), exp(.2ls)exp(.2ld)), so softmax runs in the linear
    domain with node-level exps only (P1,P2 per src / Q1,Q2 per dst).
  - Layer 2 aggregates in the 64-dim projected space: h2 = elu(out1) @ W2 is
    computed node-level (PE transpose + matmul), so the second gather rows
    are only 256B.
"""

import numpy as np
import ml_dtypes

import concourse.bacc as bacc
import concourse.mybir as mybir
import concourse.tile as tile

F32 = mybir.dt.float32
BF16 = mybir.dt.bfloat16
I16 = mybir.dt.int16
AF = mybir.ActivationFunctionType
ALU = mybir.AluOpType

NEG_SLOPE = 0.2
P = 128


class Cfg:
    def __init__(self, N, E, in_dim, hid, heads, out_dim, n_cores=8):
        self.N, self.E = N, E
        self.in_dim, self.hid, self.heads, self.out_dim = in_dim, hid, heads, out_dim
        self.n_cores = n_cores
        self.hh = hid * heads                      # layer-1 output channels
        assert N % n_cores == 0
        self.n_real = N // n_cores                 # real lanes per core
        self.L = ((self.n_real + P) // P) * P      # padded lanes (incl dead)
        assert self.n_real < self.L, "need dead lanes for gather padding"
        self.B = self.L // P                       # dst blocks per core
        self.TAB = n_cores * self.L                # table rows
        self.HIB = max(self.TAB - 32768, 0)        # hi-view base row
        self.row1 = (self.hh * 2 + 4 * self.heads * 2 + 255) // 256 * 128
        self.row1 = 384                            # 256 bf16 h + 16 f32 P + pad
        self.row2 = 128                            # 64 bf16 h2 + 2 f32 P + pad
        self.pad_lo = self.n_real                  # core0 dead lane pid
        self.pad_hi = (n_cores - 1) * self.L + self.n_real - self.HIB


FULL = Cfg(50000, 800000, 256, 32, 8, 64)


# --------------------------------------------------------------------------
# host-side graph preprocessing (index manipulation only)
# --------------------------------------------------------------------------

def preprocess(edge_index, cfg: Cfg):
    N, C = cfg.N, cfg.n_cores
    src = np.asarray(edge_index[0], np.int64)
    dst = np.asarray(edge_index[1], np.int64)
    # self-loops are handled residently on-device; edge lists exclude them
    deg = np.bincount(dst, minlength=N)

    # snake-deal nodes to cores by degree; lane = deal round (degree-sorted)
    order = np.argsort(-deg, kind="stable")
    rounds = np.arange(N) // C
    j = np.arange(N) % C
    core_sorted = np.where(rounds % 2 == 0, j, C - 1 - j)
    core_of = np.empty(N, np.int64)
    lane_of = np.empty(N, np.int64)
    core_of[order] = core_sorted
    lane_of[order] = rounds
    pid = core_of * cfg.L + lane_of

    sp = pid[src]
    ecore = core_of[dst]
    eblk = lane_of[dst] // P
    elane = lane_of[dst] % P
    lane_key = (ecore * cfg.B + eblk) * P + elane

    # per-edge side choice: forced lo (sp < HIB), forced hi (sp >= 32768),
    # free in between.  Per lane, fill lo up to a per-lane target that
    # balances the two sides as well as the forced counts allow.
    forced_hi = sp >= 32768
    is_free = (~forced_hi) & (sp >= cfg.HIB)
    NL = C * cfg.B * P
    tot = np.bincount(lane_key, minlength=NL)
    flo = np.bincount(lane_key[(~forced_hi) & (~is_free)], minlength=NL)
    ffr = np.bincount(lane_key[is_free], minlength=NL)
    # per-block optimal (D_lo, D_hi): sweep D_lo, maximize per-lane lo fill,
    # pick the split minimizing total padded slots
    lo_target = np.zeros(NL, np.int64)
    blk_of = (np.arange(NL) // P) % cfg.B
    for b in range(cfg.B):
        sel = blk_of == b
        fl, fr, tt = flo[sel], ffr[sel], tot[sel]
        best = None
        for DLc in range(int(fl.max(initial=0)), int(tt.max(initial=0)) + 1):
            lo = np.minimum(np.minimum(DLc, fl + fr), tt)
            DH = int((tt - lo).max(initial=0))
            if best is None or DLc + DH < best[0]:
                best = (DLc + DH, DLc, lo)
        lo_target[sel] = best[2] if best else 0

    # order edges per lane: forced-lo, free, forced-hi; cum position in lane
    cls = np.where(forced_hi, 2, np.where(is_free, 1, 0))
    o = np.lexsort((cls, lane_key))
    ks = lane_key[o]
    if len(ks):
        first = np.r_[0, np.where(np.diff(ks) != 0)[0] + 1]
        firsts = np.zeros(len(ks), np.int64)
        firsts[first] = first
        np.maximum.accumulate(firsts, out=firsts)
        cum = np.arange(len(ks)) - firsts
    else:
        cum = np.array([], np.int64)
    lo_t_e = lo_target[ks]
    side_hi = cum >= lo_t_e
    slot = np.where(side_hi, cum - lo_t_e, cum)
    sp_o = sp[o]
    assert not (side_hi & (sp_o < cfg.HIB)).any()
    assert not ((~side_hi) & (sp_o >= 32768)).any()

    lo_cnt = lo_target.reshape(C, cfg.B, P)
    hi_cnt = (tot - lo_target).reshape(C, cfg.B, P)
    D_lo = lo_cnt.max(axis=(0, 2)).astype(np.int64)
    D_hi = hi_cnt.max(axis=(0, 2)).astype(np.int64)

    def build_side(hi_side, D, pad_idx, base):
        m = side_hi == hi_side
        ksm = ks[m]
        cores_s = ksm // (cfg.B * P)
        blks_s = (ksm // P) % cfg.B
        lanes_s = ksm % P
        vals = sp_o[m] - base
        assert len(vals) == 0 or (vals.min() >= 0 and vals.max() < 32768)
        widths = 8 * D + 1  # int16 columns (of 16) per gather call
        offs = np.r_[0, np.cumsum(widths)].astype(np.int64)
        WT = int(offs[-1])
        out = []
        for c in range(C):
            arr = np.full((16, WT), pad_idx, np.int64)
            mm = cores_s == c
            flat = offs[blks_s[mm]] * 16 + slot[m][mm] * P + lanes_s[mm]
            arr[flat % 16, flat // 16] = vals[mm]
            out.append(np.tile(arr, (8, 1)).astype(np.int16))
        return out, offs

    idx_lo, offs_lo = build_side(False, D_lo, cfg.pad_lo, 0)
    idx_hi, offs_hi = build_side(True, D_hi, cfg.pad_hi, cfg.HIB)

    perm = np.full(C * cfg.L, -1, np.int64)
    perm[pid] = np.arange(N)

    return dict(
        pid=pid, perm=perm, D_lo=D_lo, D_hi=D_hi,
        idx_lo=idx_lo, idx_hi=idx_hi, offs_lo=offs_lo, offs_hi=offs_hi,
    )


# --------------------------------------------------------------------------
# device program
# --------------------------------------------------------------------------

def build_program(cfg: Cfg, prep):
    c = cfg
    D_lo, D_hi = prep["D_lo"], prep["D_hi"]
    offs_lo, offs_hi = prep["offs_lo"], prep["offs_hi"]
    WLO, WHI = int(offs_lo[-1]), int(offs_hi[-1])
    HH, OD = c.hh, c.out_dim
    HE, HD = c.heads, c.hid

    nc = bacc.Bacc("TRN2", num_swdge_queues=2)
    xT = nc.dram_tensor("xT", [c.in_dim, c.L], F32, kind="ExternalInput")
    wpack = nc.dram_tensor("wpack", [c.in_dim, HH + 2 * HE], F32, kind="ExternalInput")
    w2bf = nc.dram_tensor("w2bf", [HH, OD], BF16, kind="ExternalInput")
    b1b = nc.dram_tensor("b1b", [P, HH], F32, kind="ExternalInput")
    v2u2 = nc.dram_tensor("v2u2", [P, 2 * HH], F32, kind="ExternalInput")
    b2b = nc.dram_tensor("b2b", [P, OD], F32, kind="ExternalInput")
    ident = nc.dram_tensor("ident", [P, P], F32, kind="ExternalInput")
    mask_d = nc.dram_tensor("mask1", [P, 1], F32, kind="ExternalInput")
    madd_d = nc.dram_tensor("madd1", [P, 1], F32, kind="ExternalInput")
    ixlo_d = nc.dram_tensor("idx_lo", [P, WLO], I16, kind="ExternalInput")
    ixhi_d = nc.dram_tensor("idx_hi", [P, WHI], I16, kind="ExternalInput")
    out_d = nc.dram_tensor("out", [c.L, OD], F32, kind="ExternalOutput")

    t1s = nc.dram_tensor("t1s", [c.L, c.row1], BF16)
    T1 = nc.dram_tensor("T1", [c.TAB, c.row1], BF16, addr_space="Shared")
    t2s = nc.dram_tensor("t2s", [c.L, c.row2], BF16)
    T2 = nc.dram_tensor("T2", [c.TAB, c.row2], BF16, addr_space="Shared")

    KT = c.in_dim // P  # contraction tiles (2)
    NPACK = HH + 2 * HE  # 272

    with tile.TileContext(nc) as tc:
        with (
            tc.tile_pool(name="res", bufs=1) as res,
            tc.tile_pool(name="pmm", bufs=2, space="PSUM") as pmm,
            tc.tile_pool(name="ptp", bufs=2, space="PSUM") as ptp,
            tc.tile_pool(name="ph2", bufs=2, space="PSUM") as ph2p,
            tc.tile_pool(name="t1p", bufs=3) as t1p,
        ):
            # ---- resident loads
            wps = res.tile([P, KT, NPACK], F32)
            nc.sync.dma_start(out=wps[:], in_=wpack.ap().rearrange("(k p) n -> p k n", p=P))
            w2s = res.tile([P, KT, OD], BF16)
            nc.sync.dma_start(out=w2s[:], in_=w2bf.ap().rearrange("(k p) n -> p k n", p=P))
            b1s = res.tile([P, HH], F32)
            nc.sync.dma_start(out=b1s[:], in_=b1b.ap())
            vus = res.tile([P, 2, HH], F32)
            nc.sync.dma_start(out=vus[:], in_=v2u2.ap().rearrange("p (t n) -> p t n", n=HH))
            b2s = res.tile([P, OD], F32)
            nc.sync.dma_start(out=b2s[:], in_=b2b.ap())
            ids = res.tile([P, P], F32)
            nc.sync.dma_start(out=ids[:], in_=ident.ap())
            msks = res.tile([P, 1], F32)
            nc.sync.dma_start(out=msks[:], in_=mask_d.ap())
            mads = res.tile([P, 1], F32)
            nc.sync.dma_start(out=mads[:], in_=madd_d.ap())
            ixlo = res.tile([P, WLO], I16)
            nc.sync.dma_start(out=ixlo[:], in_=ixlo_d.ap())
            ixhi = res.tile([P, WHI], I16)
            nc.sync.dma_start(out=ixhi[:], in_=ixhi_d.ap())
            q12 = res.tile([P, c.B, 2 * HE], F32)    # layer-1 Q1|Q2 per block
            q2r = res.tile([P, c.B, 2], F32)         # layer-2 Q1|Q2
            hres = res.tile([P, c.B, HH], BF16)      # own h rows (self-loops)
            p12r = res.tile([P, c.B, 2 * HE], F32)   # own P1|P2
            h2res = res.tile([P, c.B, OD], BF16)     # own h2 rows
            p2r = res.tile([P, c.B, 2], F32)         # own layer-2 P1|P2

            # ================= phase 0: node tables =================
            with tc.tile_pool(name="p0res", bufs=1) as p0res:
                xts = p0res.tile([P, KT, c.L], F32)
                nc.sync.dma_start(out=xts[:], in_=xT.ap().rearrange("(k p) l -> p k l", p=P))
                for b in range(c.B):
                    ps = pmm.tile([P, NPACK], F32, tag="p0")
                    for k in range(KT):
                        nc.tensor.matmul(
                            ps[:], xts[:, k, b * P:(b + 1) * P], wps[:, k, :],
                            start=(k == 0), stop=(k == KT - 1),
                        )
                    t1t = t1p.tile([P, c.row1], BF16, tag="t1")
                    t1f = t1t[:].bitcast(F32)  # [P, row1//2]
                    hw1 = HH // 2  # f32 col of P1
                    nc.vector.tensor_copy(out=t1t[:, 0:HH], in_=ps[:, 0:HH])
                    nc.scalar.activation(t1f[:, hw1:hw1 + HE], ps[:, HH:HH + HE], AF.Exp)
                    nc.scalar.activation(t1f[:, hw1 + HE:hw1 + 2 * HE], ps[:, HH:HH + HE],
                                         AF.Exp, scale=NEG_SLOPE)
                    nc.scalar.activation(q12[:, b, 0:HE], ps[:, HH + HE:HH + 2 * HE], AF.Exp)
                    nc.scalar.activation(q12[:, b, HE:2 * HE], ps[:, HH + HE:HH + 2 * HE],
                                         AF.Exp, scale=NEG_SLOPE)
                    if b == c.B - 1:
                        nc.vector.tensor_tensor(
                            out=t1f[:, hw1:hw1 + 2 * HE], in0=t1f[:, hw1:hw1 + 2 * HE],
                            in1=msks[:].to_broadcast([P, 2 * HE]), op=ALU.mult)
                    nc.vector.tensor_copy(out=hres[:, b, :], in_=ps[:, 0:HH])
                    nc.vector.tensor_copy(out=p12r[:, b, :], in_=t1f[:, hw1:hw1 + 2 * HE])
                    nc.sync.dma_start(out=t1s[b * P:(b + 1) * P, :], in_=t1t[:])

            # ---- AllGather layer-1 table
            nc.gpsimd.collective_compute(
                "AllGather", ALU.bypass,
                replica_groups=[list(range(c.n_cores))],
                ins=[t1s.ap()], outs=[T1.ap()],
            )

            with (
                tc.tile_pool(name="gat", bufs=3) as gat,
                tc.tile_pool(name="gat2", bufs=2) as gat2,
                tc.tile_pool(name="work", bufs=2) as wk,
                tc.tile_pool(name="outp", bufs=3) as outp,
            ):
                def gather(tag, view, ix, off, D, row, qn, pool=None):
                    g = (pool or gat).tile([P, D + 1, row], BF16, tag=tag)
                    n = P * D + 16
                    nc.gpsimd.dma_gather(
                        g[:], view, ix[:, off:off + 8 * D + 1], n, n, row,
                        single_packet=n <= 1024, queue_num=qn,
                    )
                    return g

                def u_chain(g, b, D, hw, qtile, nq, tagp):
                    """u weights [P, D, nq, 1] bf16 from gathered P and resident Q."""
                    gf = g[:].bitcast(F32)
                    ut = wk.tile([P, D, 2 * nq], F32, tag=f"ut{tagp}")
                    nc.vector.tensor_tensor(
                        out=ut[:], in0=gf[:, 0:D, hw:hw + 2 * nq],
                        in1=qtile[:, b:b + 1, :].to_broadcast([P, D, 2 * nq]),
                        op=ALU.mult)
                    u = wk.tile([P, D, nq, 1], BF16, tag=f"u{tagp}")
                    nc.vector.tensor_tensor(
                        out=u[:, :, :, 0], in0=ut[:, :, 0:nq], in1=ut[:, :, nq:2 * nq],
                        op=ALU.max)
                    return u

                # ================= phase 1: layer-1 edges =================
                t1lo = T1[0:min(c.TAB, 32768), :]
                t1hi = T1[c.HIB:c.TAB, :]
                for b in range(c.B):
                    sides = []
                    for sd, (D, off, view, ix, qn) in enumerate([
                        (int(D_lo[b]), int(offs_lo[b]), t1lo, ixlo, 0),
                        (int(D_hi[b]), int(offs_hi[b]), t1hi, ixhi, 1),
                    ]):
                        if D == 0:
                            continue
                        g = gather(f"g{sd}", view, ix, off, D, c.row1, qn)
                        u = u_chain(g, b, D, HH // 2, q12, HE, sd)
                        nc.vector.tensor_tensor(
                            out=g[:, 0:D, 0:HH].rearrange("p d (h ch) -> p d h ch", ch=HD),
                            in0=u[:].to_broadcast([P, D, HE, HD]),
                            in1=g[:, 0:D, 0:HH].rearrange("p d (h ch) -> p d h ch", ch=HD),
                            op=ALU.mult)
                        sp_ = wk.tile([P, HE], F32, tag=f"sp{sd}")
                        nc.vector.tensor_reduce(
                            out=sp_[:], in_=u[:].rearrange("p d h o -> p h (d o)"),
                            axis=mybir.AxisListType.X, op=ALU.add)
                        ap_ = wk.tile([P, HH], F32, tag=f"ap{sd}")
                        nc.vector.tensor_reduce(
                            out=ap_[:], in_=g[:, 0:D, 0:HH].rearrange("p d n -> p n d"),
                            axis=mybir.AxisListType.X, op=ALU.add)
                        sides.append((sp_, ap_))
                    s = wk.tile([P, HE], F32, tag="s")
                    agg = wk.tile([P, HH], F32, tag="agg")
                    if len(sides) == 2:
                        nc.vector.tensor_tensor(out=s[:], in0=sides[0][0][:], in1=sides[1][0][:], op=ALU.add)
                        nc.vector.tensor_tensor(out=agg[:], in0=sides[0][1][:], in1=sides[1][1][:], op=ALU.add)
                    elif len(sides) == 1:
                        nc.vector.tensor_copy(out=s[:], in_=sides[0][0][:])
                        nc.vector.tensor_copy(out=agg[:], in_=sides[0][1][:])
                    else:
                        nc.vector.memset(s[:], 0)
                        nc.vector.memset(agg[:], 0)
                    # self-loop contribution (resident, no gather)
                    usl = wk.tile([P, 2 * HE], F32, tag="usl")
                    nc.vector.tensor_tensor(out=usl[:], in0=p12r[:, b, :], in1=q12[:, b, :], op=ALU.mult)
                    us = wk.tile([P, HE, 1], F32, tag="us")
                    nc.vector.tensor_tensor(out=us[:, :, 0], in0=usl[:, 0:HE], in1=usl[:, HE:2 * HE], op=ALU.max)
                    nc.vector.tensor_tensor(out=s[:], in0=s[:], in1=us[:, :, 0], op=ALU.add)
                    nc.vector.tensor_tensor(
                        out=hres[:, b, :].rearrange("p (h ch) -> p h ch", ch=HD),
                        in0=us[:].to_broadcast([P, HE, HD]),
                        in1=hres[:, b, :].rearrange("p (h ch) -> p h ch", ch=HD), op=ALU.mult)
                    nc.vector.tensor_tensor(out=agg[:], in0=agg[:], in1=hres[:, b, :], op=ALU.add)

                    if b == c.B - 1:
                        nc.vector.tensor_tensor(
                            out=s[:], in0=s[:],
                            in1=mads[:].to_broadcast([P, HE]), op=ALU.add)
                    rec = wk.tile([P, HE, 1], F32, tag="rec")
                    nc.vector.reciprocal(rec[:, :, 0], s[:])
                    o1 = wk.tile([P, HH], F32, tag="o1")
                    nc.vector.tensor_tensor(
                        out=o1[:].rearrange("p (h ch) -> p h ch", ch=HD),
                        in0=agg[:].rearrange("p (h ch) -> p h ch", ch=HD),
                        in1=rec[:].to_broadcast([P, HE, HD]), op=ALU.mult)
                    nc.vector.tensor_tensor(out=o1[:], in0=o1[:], in1=b1s[:], op=ALU.add)
                    # elu(o1) = relu(o1) + exp(o1 - relu(o1)) - 1
                    rl = wk.tile([P, HH], F32, tag="rl")
                    nc.scalar.activation(rl[:], o1[:], AF.Relu)
                    ng = wk.tile([P, HH], F32, tag="ng")
                    nc.vector.tensor_tensor(out=ng[:], in0=o1[:], in1=rl[:], op=ALU.subtract)
                    nc.scalar.activation(ng[:], ng[:], AF.Exp)
                    el = wk.tile([P, 1, HH], F32, tag="el")
                    nc.vector.tensor_tensor(out=el[:, 0, :], in0=rl[:], in1=ng[:], op=ALU.add)
                    nc.vector.tensor_scalar_add(el[:, 0, :], el[:, 0, :], -1.0)
                    # ls2 / ld2
                    d2 = wk.tile([P, 2, HH], F32, tag="d2")
                    nc.vector.tensor_tensor(
                        out=d2[:], in0=el[:].to_broadcast([P, 2, HH]), in1=vus[:], op=ALU.mult)
                    ll = wk.tile([P, 2], F32, tag="ll")
                    nc.vector.tensor_reduce(out=ll[:], in_=d2[:], axis=mybir.AxisListType.X,
                                            op=ALU.add)
                    t2t = t1p.tile([P, c.row2], BF16, tag="t2")
                    t2f = t2t[:].bitcast(F32)
                    h2w = OD // 2
                    nc.scalar.activation(t2f[:, h2w:h2w + 1], ll[:, 0:1], AF.Exp)
                    nc.scalar.activation(t2f[:, h2w + 1:h2w + 2], ll[:, 0:1], AF.Exp,
                                         scale=NEG_SLOPE)
                    nc.scalar.activation(q2r[:, b, 0:1], ll[:, 1:2], AF.Exp)
                    nc.scalar.activation(q2r[:, b, 1:2], ll[:, 1:2], AF.Exp, scale=NEG_SLOPE)
                    # h2 = el @ W2 (PE transpose path)
                    ph2 = ph2p.tile([P, OD], F32, tag="ph2")
                    for k in range(KT):
                        tp = ptp.tile([P, P], F32, tag="tp")
                        nc.tensor.transpose(tp[:], el[:, 0, k * P:(k + 1) * P], ids[:])
                        eb = wk.tile([P, P], BF16, tag="eb")
                        nc.vector.tensor_copy(out=eb[:], in_=tp[:])
                        nc.tensor.matmul(ph2[:], eb[:], w2s[:, k, :],
                                         start=(k == 0), stop=(k == KT - 1))
                    nc.vector.tensor_copy(out=t2t[:, 0:OD], in_=ph2[:])
                    if b == c.B - 1:
                        nc.vector.tensor_tensor(
                            out=t2t[:], in0=t2t[:],
                            in1=msks[:].to_broadcast([P, c.row2]), op=ALU.mult)
                    nc.vector.tensor_copy(out=h2res[:, b, :], in_=t2t[:, 0:OD])
                    nc.vector.tensor_copy(out=p2r[:, b, :], in_=t2f[:, h2w:h2w + 2])
                    nc.sync.dma_start(out=t2s[b * P:(b + 1) * P, :], in_=t2t[:])

                # ---- AllGather layer-2 table
                nc.gpsimd.collective_compute(
                    "AllGather", ALU.bypass,
                    replica_groups=[list(range(c.n_cores))],
                    ins=[t2s.ap()], outs=[T2.ap()],
                )

                # ================= phase 2: layer-2 edges =================
                t2lo = T2[0:min(c.TAB, 32768), :]
                t2hi = T2[c.HIB:c.TAB, :]
                for b in range(c.B):
                    sides = []
                    for sd, (D, off, view, ix, qn) in enumerate([
                        (int(D_lo[b]), int(offs_lo[b]), t2lo, ixlo, 0),
                        (int(D_hi[b]), int(offs_hi[b]), t2hi, ixhi, 1),
                    ]):
                        if D == 0:
                            continue
                        g = gather(f"h{sd}", view, ix, off, D, c.row2, qn, pool=gat2)
                        u = u_chain(g, b, D, OD // 2, q2r, 1, 2 + sd)
                        nc.vector.tensor_tensor(
                            out=g[:, 0:D, 0:OD],
                            in0=u[:, :, 0, :].to_broadcast([P, D, OD]),
                            in1=g[:, 0:D, 0:OD], op=ALU.mult)
                        sp_ = wk.tile([P, 1], F32, tag=f"s2p{sd}")
                        nc.vector.tensor_reduce(
                            out=sp_[:], in_=u[:].rearrange("p d h o -> p h (d o)"),
                            axis=mybir.AxisListType.X, op=ALU.add)
                        ap_ = wk.tile([P, OD], F32, tag=f"a2p{sd}")
                        nc.vector.tensor_reduce(
                            out=ap_[:], in_=g[:, 0:D, 0:OD].rearrange("p d n -> p n d"),
                            axis=mybir.AxisListType.X, op=ALU.add)
                        sides.append((sp_, ap_))
                    s = wk.tile([P, 1], F32, tag="s2")
                    agg = wk.tile([P, OD], F32, tag="agg2")
                    if len(sides) == 2:
                        nc.vector.tensor_tensor(out=s[:], in0=sides[0][0][:], in1=sides[1][0][:], op=ALU.add)
                        nc.vector.tensor_tensor(out=agg[:], in0=sides[0][1][:], in1=sides[1][1][:], op=ALU.add)
                    elif len(sides) == 1:
                        nc.vector.tensor_copy(out=s[:], in_=sides[0][0][:])
                        nc.vector.tensor_copy(out=agg[:], in_=sides[0][1][:])
                    else:
                        nc.vector.memset(s[:], 0)
                        nc.vector.memset(agg[:], 0)
                    usl2 = wk.tile([P, 2], F32, tag="usl2")
                    nc.vector.tensor_tensor(out=usl2[:], in0=p2r[:, b, :], in1=q2r[:, b, :], op=ALU.mult)
                    us2 = wk.tile([P, 1], F32, tag="us2")
                    nc.vector.tensor_tensor(out=us2[:], in0=usl2[:, 0:1], in1=usl2[:, 1:2], op=ALU.max)
                    nc.vector.tensor_tensor(out=s[:], in0=s[:], in1=us2[:], op=ALU.add)
                    nc.vector.tensor_tensor(
                        out=h2res[:, b, :], in0=us2[:].to_broadcast([P, OD]),
                        in1=h2res[:, b, :], op=ALU.mult)
                    nc.vector.tensor_tensor(out=agg[:], in0=agg[:], in1=h2res[:, b, :], op=ALU.add)
                    if b == c.B - 1:
                        nc.vector.tensor_tensor(
                            out=s[:], in0=s[:], in1=mads[:], op=ALU.add)
                    rec = wk.tile([P, 1], F32, tag="rec2")
                    nc.vector.reciprocal(rec[:], s[:])
                    o2 = outp.tile([P, OD], F32, tag="o2")
                    nc.vector.tensor_tensor(
                        out=o2[:], in0=agg[:],
                        in1=rec[:].to_broadcast([P, OD]), op=ALU.mult)
                    nc.vector.tensor_tensor(out=o2[:], in0=o2[:], in1=b2s[:], op=ALU.add)
                    nc.sync.dma_start(out=out_d[b * P:(b + 1) * P, :], in_=o2[:])

    nc.compile()
    return nc


# --------------------------------------------------------------------------
# host wrapper
# --------------------------------------------------------------------------

def make_inputs(x, W1, a_src1, a_dst1, b1, W2, a_src2, a_dst2, b2, cfg, prep):
    c = cfg
    HH, HE, HD = c.hh, c.heads, c.hid
    A = np.zeros((HH, HE), np.float32)
    Ad = np.zeros((HH, HE), np.float32)
    for h in range(HE):
        A[h * HD:(h + 1) * HD, h] = a_src1[h]
        Ad[h * HD:(h + 1) * HD, h] = a_dst1[h]
    U1 = (W1 @ A).astype(np.float32)
    V1 = (W1 @ Ad).astype(np.float32)
    wpack = np.concatenate([W1.astype(np.float32), U1, V1], axis=1)
    v2 = (W2 @ a_src2[0]).astype(np.float32)
    u2 = (W2 @ a_dst2[0]).astype(np.float32)
    v2u2 = np.tile(np.concatenate([v2, u2])[None, :], (P, 1)).astype(np.float32)
    b1b = np.tile(b1[None, :], (P, 1)).astype(np.float32)
    b2b = np.tile(b2[None, :], (P, 1)).astype(np.float32)
    ident = np.eye(P, dtype=np.float32)
    mask1 = np.ones((P, 1), np.float32)
    d0 = c.n_real % P
    mask1[d0:, 0] = 0.0
    madd1 = 1.0 - mask1
    w2bf = W2.astype(ml_dtypes.bfloat16)

    perm = prep["perm"]
    in_maps = []
    for k in range(c.n_cores):
        pk = perm[k * c.L:(k + 1) * c.L]
        xs = np.zeros((c.L, c.in_dim), np.float32)
        real = pk >= 0
        xs[real] = x[pk[real]]
        in_maps.append({
            "xT": np.ascontiguousarray(xs.T),
            "wpack": wpack, "w2bf": w2bf, "b1b": b1b, "v2u2": v2u2,
            "b2b": b2b, "ident": ident, "mask1": mask1, "madd1": madd1,
            "idx_lo": prep["idx_lo"][k], "idx_hi": prep["idx_hi"][k],
        })
    return in_maps


def unshard(results, cfg, prep):
    c = cfg
    full = np.concatenate([r["out"] for r in results], axis=0)  # [C*L, OD]
    return full[prep["pid"]].astype(np.float32)


_CACHE = {}


def run(inputs, cfg=FULL):
    edge_index = np.asarray(inputs["edge_index"])
    if cfg.N not in _CACHE:
        prep = preprocess(edge_index, cfg)
        nc = build_program(cfg, prep)
        _CACHE[cfg.N] = (prep, nc)
    prep, nc = _CACHE[cfg.N]
    in_maps = make_inputs(
        np.asarray(inputs["x"], np.float32), np.asarray(inputs["W1"], np.float32),
        np.asarray(inputs["a_src1"], np.float32), np.asarray(inputs["a_dst1"], np.float32),
        np.asarray(inputs["b1"], np.float32), np.asarray(inputs["W2"], np.float32),
        np.asarray(inputs["a_src2"], np.float32), np.asarray(inputs["a_dst2"], np.float32),
        np.asarray(inputs["b2"], np.float32), cfg, prep)
    from concourse.bass_utils import run_bass_kernel_spmd
    last = None
    for attempt in range(3):
        try:
            res = run_bass_kernel_spmd(nc, in_maps, core_ids=list(range(cfg.n_cores)))
            return unshard(res.results, cfg, prep)
        except Exception as e:  # transient tunnel/device hiccups
            last = e
            import time as _t
            _t.sleep(10)
    raise last


def kernel(**inputs) -> np.ndarray:
    return run(inputs, FULL)
